# revision 26
# baseline (speedup 1.0000x reference)
"""Trainium2 Bass kernel: causal multi-head attention (B=2, N=2048, DIM=1024, H=16, DH=64).

Sharding over 8 NeuronCores: data-parallel on batch (2) x tensor-parallel on
head groups (4 heads / core).  Each core computes Q/K/V projections for its 4
heads, causal flash-style attention, and a partial output projection against
its slice of Wo.  The 4 partial outputs per batch are summed to form the full
output.

Layout notes (per core):
  - x arrives pre-transposed and pre-cast from the host as xt = bf16(x[b].T)
    (DIM, N) so the contraction dim of every projection matmul sits on SBUF
    partitions and the load is half the bytes.
  - Q^T / K^T are kept with head-dim on partitions: pair tensors (128, 2, N)
    where partitions 0:64 hold head 2p and 64:128 hold head 2p+1.  The two
    heads of a pair issue row-tiled (tile_position) matmuls that can run
    concurrently on the PE array (K=64 each).
  - Scores are computed transposed: S^T (k_seq on partitions, q on free), so
    softmax needs no max subtraction (scores ~ N(0,1)) and P^T feeds the
    P@V matmul directly with K=128.  Row sums l come for free from a ones
    column appended to V (lhsT = [V | 1], out rows 0:64 = O^T, row 64 = l).
  - The q-chunk loop is outermost so that softmax normalization and the
    output projection of chunk c pipeline with the attention of chunk c+1
    (avoids a serialized tail that lets the PE HAM clock-gate go cold).
"""

import numpy as np
import ml_dtypes

import concourse.bass as bass
import concourse.bacc as bacc
import concourse.tile as tile
from concourse import mybir
from concourse.bass_utils import run_bass_kernel_spmd

B, N, DIM, H, DH = 2, 2048, 1024, 16, 64
HG = 4                  # heads per core
GROUPS = 4              # tensor-parallel degree (head groups)
GCOLS = HG * DH         # 256 inner columns per core
NKT = DIM // 128        # 8 contraction tiles for projections
NQC = N // 512          # 4 query chunks
NMT = N // 128          # 16 sequence tiles
SCALE = DH ** -0.5

F32 = mybir.dt.float32
BF16 = mybir.dt.bfloat16


def build():
    nc = bacc.Bacc("TRN2", target_bir_lowering=False, debug=True)

    xt = nc.declare_dram_parameter("xt", [DIM, N], BF16, isOutput=False)
    wq = nc.declare_dram_parameter("wq", [DIM, GCOLS], BF16, isOutput=False)
    wk = nc.declare_dram_parameter("wk", [DIM, GCOLS], BF16, isOutput=False)
    wv = nc.declare_dram_parameter("wv", [DIM, GCOLS], BF16, isOutput=False)
    wo = nc.declare_dram_parameter("wo", [GCOLS, DIM], BF16, isOutput=False)
    bo = nc.declare_dram_parameter("bo", [DIM], F32, isOutput=False)
    out = nc.declare_dram_parameter("out", [N, DIM], F32, isOutput=True)

    with tile.TileContext(nc) as tc:
        with (
            tc.tile_pool(name="const", bufs=1) as const,
            tc.tile_pool(name="ptp", bufs=3) as ptp,
            tc.tile_pool(name="lp", bufs=2) as lp,
            tc.tile_pool(name="outs", bufs=3) as outs,
            tc.tile_pool(name="psS", bufs=2, space="PSUM") as psS,
            tc.tile_pool(name="psO", bufs=1, space="PSUM") as psO,
            tc.tile_pool(name="psP", bufs=2, space="PSUM") as psP,
            tc.tile_pool(name="dramp", bufs=1, space="DRAM") as dramp,
        ):
            # ---------------- persistent tiles ----------------
            xT = const.tile([128, NKT, N], BF16)          # x^T (dim on partitions)
            wqsb = const.tile([128, NKT, GCOLS], BF16)
            wksb = const.tile([128, NKT, GCOLS], BF16)
            wvsb = const.tile([128, NKT, GCOLS], BF16)
            wosb = const.tile([128, 2, DIM], BF16)        # Wo rows, head-pair layout
            bosb = const.tile([128, DIM], F32)            # bo broadcast to 128 parts
            tmask = const.tile([128, 2, 128], BF16)       # triangular binary mask
            v1 = const.tile([128, NMT, HG, DH + 1], BF16)  # [V | ones]
            qth = const.tile([128, 2, N], BF16)           # Q^T head pairs
            kth = const.tile([128, 2, N], BF16)           # K^T head pairs
            ost = const.tile([65, HG, N], F32)            # unnormalized O^T + l row
            sel = const.tile([16, 16, 64], BF16)          # PE-broadcast selectors
            osb = const.tile([128, 2, N], BF16)           # normalized O^T, pair layout

            # ---------------- phase 1: loads (HWDGE, already bf16) ----------
            # wk first (the first projection emitted is K^T), then x per
            # k-tile so the K^T matmuls can start as soon as tiles land.
            nc.sync.dma_start(out=wksb[:, :, :],
                              in_=wk[:, :].rearrange("(t p) n -> p t n", p=128))
            for k in range(NKT):
                nc.sync.dma_start(out=xT[:, k, :], in_=xt[k * 128:(k + 1) * 128, :])
            for wsb, wdr in ((wqsb, wq), (wvsb, wv)):
                nc.sync.dma_start(
                    out=wsb[:, :, :],
                    in_=wdr[:, :].rearrange("(t p) n -> p t n", p=128),
                )
            for h in range(HG):
                p, e = divmod(h, 2)
                nc.sync.dma_start(out=wosb[e * 64:(e + 1) * 64, p, :],
                                  in_=wo[h * DH:(h + 1) * DH, :])
            bo_ap = bo[:]
            bo_bcast = bass.AP(tensor=bo_ap.tensor, offset=bo_ap.offset,
                               ap=[[0, 128]] + list(bo_ap.ap))
            nc.sync.dma_start(out=bosb[:, :], in_=bo_bcast)

            # ones column of [V | 1]
            nc.vector.memset(v1[:, :, :, DH:DH + 1], 1.0)
            # triangular binary mask for the 128-wide diagonal boundary
            # sub-block (identical for every diagonal block): keep q >= k,
            # i.e. f - p >= 0
            nc.gpsimd.memset(tmask[:, :, :], 1.0)
            nc.gpsimd.affine_select(
                out=tmask[:, :, :], in_=tmask[:, :, :],
                compare_op=mybir.AluOpType.is_ge, fill=0.0,
                base=0, pattern=[[0, 2], [1, 128]],
                channel_multiplier=-1,
            )
            # selector matrices for broadcasting softmax denominators across
            # partitions on the PE: sel[:, row, :].T @ r replicates r's row
            # `row` onto 64 output partitions
            nc.gpsimd.memset(sel[:, :, :], 1.0)
            nc.gpsimd.affine_select(
                out=sel[:, :, :], in_=sel[:, :, :],
                compare_op=mybir.AluOpType.is_equal, fill=0.0,
                base=0, pattern=[[-1, 16], [0, 64]],
                channel_multiplier=1,
            )

            # ---------- phases 2-4 fused: proj + attention per q-chunk ------
            def proj_chunk(c):
                # Q^T / K^T / V projections for chunk c's q range and k tiles
                cs = slice(c * 512, (c + 1) * 512)
                for dst, wsb in ((kth, wksb), (qth, wqsb)):
                    for pair in range(2):
                        pcols = slice(pair * 128, (pair + 1) * 128)
                        ps = psP.tile([128, 512], F32, tag="proj", name="ps_proj")
                        for k in range(NKT):
                            nc.tensor.matmul(ps[:, :], wsb[:, k, pcols], xT[:, k, cs],
                                             start=(k == 0), stop=(k == NKT - 1))
                        nc.vector.tensor_copy(out=dst[:, pair, cs], in_=ps[:, :])
                for mt in range(4 * c, 4 * c + 4):
                    ps = psP.tile([128, GCOLS], F32, tag="proj", name="ps_v")
                    for k in range(NKT):
                        nc.tensor.matmul(ps[:, :], xT[:, k, mt * 128:(mt + 1) * 128],
                                         wvsb[:, k, :],
                                         start=(k == 0), stop=(k == NKT - 1))
                    nc.vector.tensor_copy(
                        out=v1[:, mt, :, 0:DH],
                        in_=ps[:, :].rearrange("p (h d) -> p h d", h=HG),
                    )

            rdram = dramp.tile([64, 128], F32)

            def out_proj(c):
                # output projection (partial) for chunk c's sequence tiles
                for mt in range(4 * c, 4 * c + 4):
                    ms = slice(mt * 128, (mt + 1) * 128)
                    for nh in range(2):
                        ns = slice(nh * 512, (nh + 1) * 512)
                        ps = psP.tile([128, 512], F32, tag="proj", name="ps_out")
                        for p in range(2):
                            nc.tensor.matmul(ps[:, :], osb[:, p, ms], wosb[:, p, ns],
                                             start=(p == 0), stop=(p == 1))
                        ot = outs.tile([128, 512], F32, tag="ot", name="ot")
                        nc.vector.tensor_add(ot[:, :], ps[:, :], bosb[:, ns])
                        nc.sync.dma_start(out=out[ms, ns], in_=ot[:, :])

            for c in range(NQC):
                proj_chunk(c)
                qs = slice(c * 512, (c + 1) * 512)
                nkt = 4 * (c + 1)
                lc = lp.tile([16, 128], F32, tag="lc", name="lc")
                for pair in range(2):
                    hA, hB = 2 * pair, 2 * pair + 1
                    poA = psO.tile([65, 512], F32, tag="oA", name="poA")
                    poB = psO.tile([65, 512], F32, tag="oB", name="poB")
                    for k in range(nkt):
                        ks = slice(k * 128, (k + 1) * 128)
                        ss = psS.tile([128, 1024], F32, tag="s", name="ss")
                        # S^T = K^T.T @ Q^T for both heads of the pair
                        # (row-tiled: head A rows 0:64, head B rows 64:128)
                        nc.tensor.matmul(ss[:, 0:512], kth[0:64, pair, ks],
                                         qth[0:64, pair, qs], start=True, stop=True)
                        nc.tensor.matmul(ss[:, 512:1024], kth[64:128, pair, ks],
                                         qth[64:128, pair, qs], start=True, stop=True)
                        pt = ptp.tile([128, 2, 512], BF16, tag="pt", name="pt")
                        j = k - 4 * c
                        if j < 0:
                            # below the diagonal: everything unmasked
                            nc.scalar.activation(out=pt[:, :, :], in_=ss[:, :],
                                                 func=mybir.ActivationFunctionType.Exp,
                                                 scale=SCALE)
                        else:
                            # diagonal block: columns f < 128*j are fully
                            # masked (zeroed, exp skipped); columns in
                            # [128j, 128j+128) are triangular (exp then
                            # multiply by the triangular mask); the rest is
                            # unmasked
                            fs = 128 * j
                            if fs:
                                nc.vector.memset(pt[:, :, 0:fs], 0.0)
                            nc.scalar.activation(
                                out=pt[:, :, fs:], in_=ss[:, :].rearrange(
                                    "p (e f) -> p e f", e=2)[:, :, fs:],
                                func=mybir.ActivationFunctionType.Exp,
                                scale=SCALE)
                            nc.vector.tensor_mul(pt[:, :, fs:fs + 128],
                                                 pt[:, :, fs:fs + 128],
                                                 tmask[:, :, :])
                        nc.tensor.matmul(poA[:, :], v1[:, k, hA, :], pt[:, 0, :],
                                         start=(k == 0), stop=(k == nkt - 1))
                        nc.tensor.matmul(poB[:, :], v1[:, k, hB, :], pt[:, 1, :],
                                         start=(k == 0), stop=(k == nkt - 1))
                    nc.vector.tensor_copy(out=ost[:, hA, qs], in_=poA[:, :])
                    nc.vector.tensor_copy(out=ost[:, hB, qs], in_=poB[:, :])
                    # stash both heads' l rows (partition 64) into the packed
                    # per-chunk l tile, split into 4 q-subblocks of 128 per
                    # head so the reciprocal spreads over 16 DVE lanes
                    for h in (hA, hB):
                        nc.sync.dma_start(
                            out=lc[4 * h: 4 * h + 4, :],
                            in_=ost[64:65, h, qs],
                        )

                # softmax denominators for this chunk, then normalize O^T.
                # For chunks that still have attention work behind them the
                # reciprocal rows bounce through DRAM (partition-broadcast
                # DMA, zero PE cost, latency hidden by the next chunk); the
                # final chunk's chain is exposed, so it broadcasts via tiny
                # selector matmuls on the PE instead (lower latency, and the
                # PE work fills the tail bubble).
                last = c == NQC - 1
                rc32 = lp.tile([16, 128], F32, tag="rc32", name="rc32")
                nc.vector.reciprocal(out=rc32[:, :], in_=lc[:, :])
                if last:
                    rc = lp.tile([16, 128], BF16, tag="rc", name="rc")
                    nc.vector.tensor_copy(out=rc[:, :], in_=rc32[:, :])
                else:
                    nc.sync.dma_start(out=rdram[c * 16:(c + 1) * 16, :],
                                      in_=rc32[:, :])
                    rb = lp.tile([64, 16, 128], F32, tag="rb", name="rb")
                    src = rdram[c * 16:(c + 1) * 16, :]
                    bcast = bass.AP(tensor=src.tensor, offset=src.offset,
                                    ap=[[0, 64]] + list(src.ap))
                    nc.sync.dma_start(out=rb[:, :, :], in_=bcast)
                for h in range(HG):
                    p, e = divmod(h, 2)
                    if last:
                        rbp = psP.tile([64, 512], F32, tag="proj", name="rb_ps")
                        for s in range(4):
                            nc.tensor.matmul(rbp[:, s * 128:(s + 1) * 128],
                                             sel[:, 4 * h + s, :], rc[:, :],
                                             start=True, stop=True)
                        r_src = rbp[:, :].rearrange("p (s f) -> p s f", f=128)
                    else:
                        r_src = rb[:, 4 * h: 4 * h + 4, :]
                    o_src = ost[0:64, h, qs].rearrange("p (s f) -> p s f", f=128)
                    if e == 0:
                        nc.vector.tensor_mul(
                            osb[0:64, p, qs].rearrange("p (s f) -> p s f", f=128),
                            o_src, r_src)
                    else:
                        # odd head lives on partitions 64:128 of the pair
                        # tensor; DVE can't cross partitions, so stage + DMA
                        onst = outs.tile([64, 512], BF16, tag="onst", name="onst")
                        nc.vector.tensor_mul(
                            onst[:, :].rearrange("p (s f) -> p s f", f=128),
                            o_src, r_src)
                        nc.sync.dma_start(out=osb[64:128, p, qs], in_=onst[:, :])

                # out-proj lags one chunk behind so its PE work can fill the
                # normalize-chain bubble of the final chunk
                if c >= 1:
                    out_proj(c - 1)
            out_proj(NQC - 1)

    nc.compile()
    return nc


def _in_maps(inputs):
    bf = ml_dtypes.bfloat16
    x = np.asarray(inputs["x"], np.float32)
    Wq = np.asarray(inputs["Wq"], np.float32).astype(bf)
    Wkv = np.asarray(inputs["Wkv"], np.float32).astype(bf)
    Wo = np.asarray(inputs["Wo"], np.float32).astype(bf)
    bo = np.asarray(inputs["bo"], np.float32)
    maps = []
    for i in range(8):
        b, g = divmod(i, GROUPS)
        cs = slice(g * GCOLS, (g + 1) * GCOLS)
        maps.append(dict(
            xt=np.ascontiguousarray(x[b].T.astype(bf)),
            wq=np.ascontiguousarray(Wq[:, cs]),
            wk=np.ascontiguousarray(Wkv[:, cs]),
            wv=np.ascontiguousarray(Wkv[:, DIM + g * GCOLS: DIM + (g + 1) * GCOLS]),
            wo=np.ascontiguousarray(Wo[cs, :]),
            bo=np.ascontiguousarray(bo / GROUPS),
        ))
    return maps


_NC = None


def _get_nc():
    global _NC
    if _NC is None:
        nc = build()
        nc.finalize()
        _NC = nc
    return _NC


def run(inputs, trace=False, **kwargs):
    maps = _in_maps(inputs)
    res = run_bass_kernel_spmd(_get_nc(), maps, core_ids=list(range(8)),
                               trace=trace, **kwargs)
    out = np.empty((B, N, DIM), np.float32)
    for b in range(B):
        acc = res.results[4 * b]["out"].astype(np.float32).copy()
        for g in range(1, GROUPS):
            acc += res.results[4 * b + g]["out"]
        out[b] = acc
    return out, res


def kernel(**inputs):
    out, _ = run(inputs, trace=False)
    return out


# revision 27
# speedup vs baseline: 1.1210x; 1.1210x over previous
"""Trainium2 Bass kernel: causal multi-head attention (B=2, N=2048, DIM=1024, H=16, DH=64).

Sharding over 8 NeuronCores: data-parallel on batch (2) x tensor-parallel on
head groups (4 heads / core).  Each core computes Q/K/V projections for its 4
heads, causal flash-style attention, and a partial output projection against
its slice of Wo.  The 4 partial outputs per batch are summed to form the full
output.

Layout notes (per core):
  - x arrives pre-transposed and pre-cast from the host as xt = bf16(x[b].T)
    (DIM, N) so the contraction dim of every projection matmul sits on SBUF
    partitions and the load is half the bytes.
  - Q^T / K^T are kept with head-dim on partitions: pair tensors (128, 2, N)
    where partitions 0:64 hold head 2p and 64:128 hold head 2p+1.  The two
    heads of a pair issue row-tiled (tile_position) matmuls that can run
    concurrently on the PE array (K=64 each).
  - Scores are computed transposed: S^T (k_seq on partitions, q on free), so
    softmax needs no max subtraction (scores ~ N(0,1)) and P^T feeds the
    P@V matmul directly with K=128.  Row sums l come for free from a ones
    column appended to V (lhsT = [V | 1], out rows 0:64 = O^T, row 64 = l).
  - The q-chunk loop is outermost so that softmax normalization and the
    output projection of chunk c pipeline with the attention of chunk c+1
    (avoids a serialized tail that lets the PE HAM clock-gate go cold).
"""

import numpy as np
import ml_dtypes

import concourse.bass as bass
import concourse.bacc as bacc
import concourse.tile as tile
from concourse import mybir
from concourse.bass_utils import run_bass_kernel_spmd

B, N, DIM, H, DH = 2, 2048, 1024, 16, 64
HG = 4                  # heads per core
GROUPS = 4              # tensor-parallel degree (head groups)
GCOLS = HG * DH         # 256 inner columns per core
NKT = DIM // 128        # 8 contraction tiles for projections
NQC = N // 512          # 4 query chunks
NMT = N // 128          # 16 sequence tiles
SCALE = DH ** -0.5

F32 = mybir.dt.float32
BF16 = mybir.dt.bfloat16


def build():
    nc = bacc.Bacc("TRN2", target_bir_lowering=False, debug=True)

    xt = nc.declare_dram_parameter("xt", [DIM, N], BF16, isOutput=False)
    wq = nc.declare_dram_parameter("wq", [DIM, GCOLS], BF16, isOutput=False)
    wk = nc.declare_dram_parameter("wk", [DIM, GCOLS], BF16, isOutput=False)
    wv = nc.declare_dram_parameter("wv", [DIM, GCOLS], BF16, isOutput=False)
    wo = nc.declare_dram_parameter("wo", [GCOLS, DIM], BF16, isOutput=False)
    bo = nc.declare_dram_parameter("bo", [DIM], F32, isOutput=False)
    out = nc.declare_dram_parameter("out", [N, DIM], F32, isOutput=True)

    with tile.TileContext(nc) as tc:
        with (
            tc.tile_pool(name="const", bufs=1) as const,
            tc.tile_pool(name="ptp", bufs=3) as ptp,
            tc.tile_pool(name="lp", bufs=2) as lp,
            tc.tile_pool(name="outs", bufs=3) as outs,
            tc.tile_pool(name="psS", bufs=2, space="PSUM") as psS,
            tc.tile_pool(name="psO", bufs=1, space="PSUM") as psO,
            tc.tile_pool(name="psP", bufs=2, space="PSUM") as psP,
            tc.tile_pool(name="dramp", bufs=1, space="DRAM") as dramp,
        ):
            # ---------------- persistent tiles ----------------
            xT = const.tile([128, NKT, N], BF16)          # x^T (dim on partitions)
            wqsb = const.tile([128, NKT, GCOLS], BF16)
            wksb = const.tile([128, NKT, GCOLS], BF16)
            wvsb = const.tile([128, NKT, GCOLS], BF16)
            wosb = const.tile([128, 2, DIM], BF16)        # Wo rows, head-pair layout
            bosb = const.tile([128, DIM], F32)            # bo broadcast to 128 parts
            tmask = const.tile([128, 2, 128], BF16)       # triangular binary mask
            v1 = const.tile([128, NMT, HG, DH + 1], BF16)  # [V | ones]
            qth = const.tile([128, 2, N], BF16)           # Q^T head pairs
            kth = const.tile([128, 2, N], BF16)           # K^T head pairs
            ost = const.tile([65, HG, N], F32)            # unnormalized O^T + l row
            sel = const.tile([16, 16, 64], BF16)          # PE-broadcast selectors
            osb = const.tile([128, 2, N], BF16)           # normalized O^T, pair layout

            # ---------------- phase 1: loads (HWDGE, already bf16) ----------
            # wk first (the first projection emitted is K^T), then x per
            # k-tile so the K^T matmuls can start as soon as tiles land.
            nc.sync.dma_start(out=wksb[:, :, :],
                              in_=wk[:, :].rearrange("(t p) n -> p t n", p=128))
            for k in range(NKT):
                nc.sync.dma_start(out=xT[:, k, :], in_=xt[k * 128:(k + 1) * 128, :])
            for wsb, wdr in ((wqsb, wq), (wvsb, wv)):
                nc.sync.dma_start(
                    out=wsb[:, :, :],
                    in_=wdr[:, :].rearrange("(t p) n -> p t n", p=128),
                )
            for h in range(HG):
                p, e = divmod(h, 2)
                nc.sync.dma_start(out=wosb[e * 64:(e + 1) * 64, p, :],
                                  in_=wo[h * DH:(h + 1) * DH, :])
            bo_ap = bo[:]
            bo_bcast = bass.AP(tensor=bo_ap.tensor, offset=bo_ap.offset,
                               ap=[[0, 128]] + list(bo_ap.ap))
            nc.sync.dma_start(out=bosb[:, :], in_=bo_bcast)

            # ones column of [V | 1]
            nc.vector.memset(v1[:, :, :, DH:DH + 1], 1.0)
            # triangular binary mask for the 128-wide diagonal boundary
            # sub-block (identical for every diagonal block): keep q >= k,
            # i.e. f - p >= 0
            nc.gpsimd.memset(tmask[:, :, :], 1.0)
            nc.gpsimd.affine_select(
                out=tmask[:, :, :], in_=tmask[:, :, :],
                compare_op=mybir.AluOpType.is_ge, fill=0.0,
                base=0, pattern=[[0, 2], [1, 128]],
                channel_multiplier=-1,
            )
            # selector matrices for broadcasting softmax denominators across
            # partitions on the PE: sel[:, row, :].T @ r replicates r's row
            # `row` onto 64 output partitions
            nc.gpsimd.memset(sel[:, :, :], 1.0)
            nc.gpsimd.affine_select(
                out=sel[:, :, :], in_=sel[:, :, :],
                compare_op=mybir.AluOpType.is_equal, fill=0.0,
                base=0, pattern=[[-1, 16], [0, 64]],
                channel_multiplier=1,
            )

            # ---------- phases 2-4 fused: proj + attention per q-chunk ------
            def proj_chunk(c):
                # Q^T / K^T / V projections for chunk c's q range and k tiles
                cs = slice(c * 512, (c + 1) * 512)
                for dst, wsb in ((kth, wksb), (qth, wqsb)):
                    for pair in range(2):
                        pcols = slice(pair * 128, (pair + 1) * 128)
                        ps = psP.tile([128, 512], F32, tag="proj", name="ps_proj")
                        for k in range(NKT):
                            nc.tensor.matmul(ps[:, :], wsb[:, k, pcols], xT[:, k, cs],
                                             start=(k == 0), stop=(k == NKT - 1))
                        nc.vector.tensor_copy(out=dst[:, pair, cs], in_=ps[:, :])
                for mt in range(4 * c, 4 * c + 4):
                    ps = psP.tile([128, GCOLS], F32, tag="proj", name="ps_v")
                    for k in range(NKT):
                        nc.tensor.matmul(ps[:, :], xT[:, k, mt * 128:(mt + 1) * 128],
                                         wvsb[:, k, :],
                                         start=(k == 0), stop=(k == NKT - 1))
                    nc.vector.tensor_copy(
                        out=v1[:, mt, :, 0:DH],
                        in_=ps[:, :].rearrange("p (h d) -> p h d", h=HG),
                    )

            rdram = dramp.tile([64, 128], F32)

            def out_proj(c):
                # output projection (partial) for chunk c's sequence tiles
                for mt in range(4 * c, 4 * c + 4):
                    ms = slice(mt * 128, (mt + 1) * 128)
                    for nh in range(2):
                        ns = slice(nh * 512, (nh + 1) * 512)
                        ps = psP.tile([128, 512], F32, tag="proj", name="ps_out")
                        for p in range(2):
                            nc.tensor.matmul(ps[:, :], osb[:, p, ms], wosb[:, p, ns],
                                             start=(p == 0), stop=(p == 1))
                        ot = outs.tile([128, 512], F32, tag="ot", name="ot")
                        nc.vector.tensor_add(ot[:, :], ps[:, :], bosb[:, ns])
                        nc.sync.dma_start(out=out[ms, ns], in_=ot[:, :])

            for c in range(NQC):
                proj_chunk(c)
            for c in range(NQC):
                qs = slice(c * 512, (c + 1) * 512)
                nkt = 4 * (c + 1)
                lc = lp.tile([16, 128], F32, tag="lc", name="lc")
                for pair in range(2):
                    hA, hB = 2 * pair, 2 * pair + 1
                    poA = psO.tile([65, 512], F32, tag="oA", name="poA")
                    poB = psO.tile([65, 512], F32, tag="oB", name="poB")
                    for k in range(nkt):
                        ks = slice(k * 128, (k + 1) * 128)
                        ss = psS.tile([128, 1024], F32, tag="s", name="ss")
                        # S^T = K^T.T @ Q^T for both heads of the pair
                        # (row-tiled: head A rows 0:64, head B rows 64:128)
                        nc.tensor.matmul(ss[:, 0:512], kth[0:64, pair, ks],
                                         qth[0:64, pair, qs], start=True, stop=True)
                        nc.tensor.matmul(ss[:, 512:1024], kth[64:128, pair, ks],
                                         qth[64:128, pair, qs], start=True, stop=True)
                        pt = ptp.tile([128, 2, 512], BF16, tag="pt", name="pt")
                        j = k - 4 * c
                        if j < 0:
                            # below the diagonal: everything unmasked
                            nc.scalar.activation(out=pt[:, :, :], in_=ss[:, :],
                                                 func=mybir.ActivationFunctionType.Exp,
                                                 scale=SCALE)
                        else:
                            # diagonal block: columns f < 128*j are fully
                            # masked (zeroed, exp skipped); columns in
                            # [128j, 128j+128) are triangular (exp then
                            # multiply by the triangular mask); the rest is
                            # unmasked
                            fs = 128 * j
                            if fs:
                                nc.vector.memset(pt[:, :, 0:fs], 0.0)
                            nc.scalar.activation(
                                out=pt[:, :, fs:], in_=ss[:, :].rearrange(
                                    "p (e f) -> p e f", e=2)[:, :, fs:],
                                func=mybir.ActivationFunctionType.Exp,
                                scale=SCALE)
                            nc.vector.tensor_mul(pt[:, :, fs:fs + 128],
                                                 pt[:, :, fs:fs + 128],
                                                 tmask[:, :, :])
                        nc.tensor.matmul(poA[:, :], v1[:, k, hA, :], pt[:, 0, :],
                                         start=(k == 0), stop=(k == nkt - 1))
                        nc.tensor.matmul(poB[:, :], v1[:, k, hB, :], pt[:, 1, :],
                                         start=(k == 0), stop=(k == nkt - 1))
                    nc.vector.tensor_copy(out=ost[:, hA, qs], in_=poA[:, :])
                    nc.vector.tensor_copy(out=ost[:, hB, qs], in_=poB[:, :])
                    # stash both heads' l rows (partition 64) into the packed
                    # per-chunk l tile, split into 4 q-subblocks of 128 per
                    # head so the reciprocal spreads over 16 DVE lanes
                    for h in (hA, hB):
                        nc.sync.dma_start(
                            out=lc[4 * h: 4 * h + 4, :],
                            in_=ost[64:65, h, qs],
                        )

                # softmax denominators for this chunk, then normalize O^T.
                # For chunks that still have attention work behind them the
                # reciprocal rows bounce through DRAM (partition-broadcast
                # DMA, zero PE cost, latency hidden by the next chunk); the
                # final chunk's chain is exposed, so it broadcasts via tiny
                # selector matmuls on the PE instead (lower latency, and the
                # PE work fills the tail bubble).
                last = c == NQC - 1
                rc32 = lp.tile([16, 128], F32, tag="rc32", name="rc32")
                nc.vector.reciprocal(out=rc32[:, :], in_=lc[:, :])
                if last:
                    rc = lp.tile([16, 128], BF16, tag="rc", name="rc")
                    nc.vector.tensor_copy(out=rc[:, :], in_=rc32[:, :])
                else:
                    nc.sync.dma_start(out=rdram[c * 16:(c + 1) * 16, :],
                                      in_=rc32[:, :])
                    rb = lp.tile([64, 16, 128], F32, tag="rb", name="rb")
                    src = rdram[c * 16:(c + 1) * 16, :]
                    bcast = bass.AP(tensor=src.tensor, offset=src.offset,
                                    ap=[[0, 64]] + list(src.ap))
                    nc.sync.dma_start(out=rb[:, :, :], in_=bcast)
                for h in range(HG):
                    p, e = divmod(h, 2)
                    if last:
                        rbp = psP.tile([64, 512], F32, tag="proj", name="rb_ps")
                        for s in range(4):
                            nc.tensor.matmul(rbp[:, s * 128:(s + 1) * 128],
                                             sel[:, 4 * h + s, :], rc[:, :],
                                             start=True, stop=True)
                        r_src = rbp[:, :].rearrange("p (s f) -> p s f", f=128)
                    else:
                        r_src = rb[:, 4 * h: 4 * h + 4, :]
                    o_src = ost[0:64, h, qs].rearrange("p (s f) -> p s f", f=128)
                    if e == 0:
                        nc.vector.tensor_mul(
                            osb[0:64, p, qs].rearrange("p (s f) -> p s f", f=128),
                            o_src, r_src)
                    else:
                        # odd head lives on partitions 64:128 of the pair
                        # tensor; DVE can't cross partitions, so stage + DMA
                        onst = outs.tile([64, 512], BF16, tag="onst", name="onst")
                        nc.vector.tensor_mul(
                            onst[:, :].rearrange("p (s f) -> p s f", f=128),
                            o_src, r_src)
                        nc.sync.dma_start(out=osb[64:128, p, qs], in_=onst[:, :])

                # out-proj lags one chunk behind so its PE work can fill the
                # normalize-chain bubble of the final chunk
                if c >= 1:
                    out_proj(c - 1)
            out_proj(NQC - 1)

    nc.compile()
    return nc


def _in_maps(inputs):
    bf = ml_dtypes.bfloat16
    x = np.asarray(inputs["x"], np.float32)
    Wq = np.asarray(inputs["Wq"], np.float32).astype(bf)
    Wkv = np.asarray(inputs["Wkv"], np.float32).astype(bf)
    Wo = np.asarray(inputs["Wo"], np.float32).astype(bf)
    bo = np.asarray(inputs["bo"], np.float32)
    maps = []
    for i in range(8):
        b, g = divmod(i, GROUPS)
        cs = slice(g * GCOLS, (g + 1) * GCOLS)
        maps.append(dict(
            xt=np.ascontiguousarray(x[b].T.astype(bf)),
            wq=np.ascontiguousarray(Wq[:, cs]),
            wk=np.ascontiguousarray(Wkv[:, cs]),
            wv=np.ascontiguousarray(Wkv[:, DIM + g * GCOLS: DIM + (g + 1) * GCOLS]),
            wo=np.ascontiguousarray(Wo[cs, :]),
            bo=np.ascontiguousarray(bo / GROUPS),
        ))
    return maps


_NC = None


def _get_nc():
    global _NC
    if _NC is None:
        nc = build()
        nc.finalize()
        _NC = nc
    return _NC


def run(inputs, trace=False, **kwargs):
    maps = _in_maps(inputs)
    res = run_bass_kernel_spmd(_get_nc(), maps, core_ids=list(range(8)),
                               trace=trace, **kwargs)
    out = np.empty((B, N, DIM), np.float32)
    for b in range(B):
        acc = res.results[4 * b]["out"].astype(np.float32).copy()
        for g in range(1, GROUPS):
            acc += res.results[4 * b + g]["out"]
        out[b] = acc
    return out, res


def kernel(**inputs):
    out, _ = run(inputs, trace=False)
    return out


# revision 28
# speedup vs baseline: 1.1362x; 1.0136x over previous
"""Trainium2 Bass kernel: causal multi-head attention (B=2, N=2048, DIM=1024, H=16, DH=64).

Sharding over 8 NeuronCores: data-parallel on batch (2) x tensor-parallel on
head groups (4 heads / core).  Each core computes Q/K/V projections for its 4
heads, causal flash-style attention, and a partial output projection against
its slice of Wo.  The 4 partial outputs per batch are summed to form the full
output.

Layout notes (per core):
  - x arrives pre-transposed and pre-cast from the host as xt = bf16(x[b].T)
    (DIM, N) so the contraction dim of every projection matmul sits on SBUF
    partitions and the load is half the bytes.
  - Q^T / K^T are kept with head-dim on partitions: pair tensors (128, 2, N)
    where partitions 0:64 hold head 2p and 64:128 hold head 2p+1.  The two
    heads of a pair issue row-tiled (tile_position) matmuls that can run
    concurrently on the PE array (K=64 each).
  - Scores are computed transposed: S^T (k_seq on partitions, q on free), so
    softmax needs no max subtraction (scores ~ N(0,1)) and P^T feeds the
    P@V matmul directly with K=128.  Row sums l come for free from a ones
    column appended to V (lhsT = [V | 1], out rows 0:64 = O^T, row 64 = l).
  - The q-chunk loop is outermost so that softmax normalization and the
    output projection of chunk c pipeline with the attention of chunk c+1
    (avoids a serialized tail that lets the PE HAM clock-gate go cold).
"""

import numpy as np
import ml_dtypes

import concourse.bass as bass
import concourse.bacc as bacc
import concourse.tile as tile
from concourse import mybir
from concourse.bass_utils import run_bass_kernel_spmd

B, N, DIM, H, DH = 2, 2048, 1024, 16, 64
HG = 4                  # heads per core
GROUPS = 4              # tensor-parallel degree (head groups)
GCOLS = HG * DH         # 256 inner columns per core
NKT = DIM // 128        # 8 contraction tiles for projections
NQC = N // 512          # 4 query chunks
NMT = N // 128          # 16 sequence tiles
SCALE = DH ** -0.5

F32 = mybir.dt.float32
BF16 = mybir.dt.bfloat16


def build():
    nc = bacc.Bacc("TRN2", target_bir_lowering=False, debug=True)

    xt = nc.declare_dram_parameter("xt", [DIM, N], BF16, isOutput=False)
    wq = nc.declare_dram_parameter("wq", [DIM, GCOLS], BF16, isOutput=False)
    wk = nc.declare_dram_parameter("wk", [DIM, GCOLS], BF16, isOutput=False)
    wv = nc.declare_dram_parameter("wv", [DIM, GCOLS], BF16, isOutput=False)
    wo = nc.declare_dram_parameter("wo", [GCOLS, DIM], BF16, isOutput=False)
    bo = nc.declare_dram_parameter("bo", [DIM], F32, isOutput=False)
    out = nc.declare_dram_parameter("out", [N, DIM], F32, isOutput=True)

    with tile.TileContext(nc) as tc:
        with (
            tc.tile_pool(name="const", bufs=1) as const,
            tc.tile_pool(name="ptp", bufs=3) as ptp,
            tc.tile_pool(name="lp", bufs=2) as lp,
            tc.tile_pool(name="outs", bufs=3) as outs,
            tc.tile_pool(name="psS", bufs=2, space="PSUM") as psS,
            tc.tile_pool(name="psO", bufs=1, space="PSUM") as psO,
            tc.tile_pool(name="psP", bufs=2, space="PSUM") as psP,
            tc.tile_pool(name="dramp", bufs=1, space="DRAM") as dramp,
        ):
            # ---------------- persistent tiles ----------------
            xT = const.tile([128, NKT, N], BF16)          # x^T (dim on partitions)
            wqsb = const.tile([128, NKT, GCOLS], BF16)
            wksb = const.tile([128, NKT, GCOLS], BF16)
            wvsb = const.tile([128, NKT, GCOLS], BF16)
            wosb = const.tile([128, 2, DIM], BF16)        # Wo rows, head-pair layout
            bosb = const.tile([128, DIM], F32)            # bo broadcast to 128 parts
            tmask = const.tile([128, 2, 128], BF16)       # triangular binary mask
            v1 = const.tile([128, NMT, HG, DH + 1], BF16)  # [V | ones]
            qth = const.tile([128, 2, N], BF16)           # Q^T head pairs
            kth = const.tile([128, 2, N], BF16)           # K^T head pairs
            ost = const.tile([65, HG, N], F32)            # unnormalized O^T + l row
            sel = const.tile([16, 16, 64], BF16)          # PE-broadcast selectors
            osb = const.tile([128, 2, N], BF16)           # normalized O^T, pair layout

            # ---------------- phase 1: loads (HWDGE, already bf16) ----------
            # wk first (the first projection emitted is K^T), then x per
            # k-tile so the K^T matmuls can start as soon as tiles land.
            nc.sync.dma_start(out=wksb[:, :, :],
                              in_=wk[:, :].rearrange("(t p) n -> p t n", p=128))
            for k in range(NKT):
                nc.sync.dma_start(out=xT[:, k, :], in_=xt[k * 128:(k + 1) * 128, :])
            for wsb, wdr in ((wqsb, wq), (wvsb, wv)):
                nc.sync.dma_start(
                    out=wsb[:, :, :],
                    in_=wdr[:, :].rearrange("(t p) n -> p t n", p=128),
                )
            for h in range(HG):
                p, e = divmod(h, 2)
                nc.sync.dma_start(out=wosb[e * 64:(e + 1) * 64, p, :],
                                  in_=wo[h * DH:(h + 1) * DH, :])
            bo_ap = bo[:]
            bo_bcast = bass.AP(tensor=bo_ap.tensor, offset=bo_ap.offset,
                               ap=[[0, 128]] + list(bo_ap.ap))
            nc.sync.dma_start(out=bosb[:, :], in_=bo_bcast)

            # ones column of [V | 1]
            nc.vector.memset(v1[:, :, :, DH:DH + 1], 1.0)
            # triangular binary mask for the 128-wide diagonal boundary
            # sub-block (identical for every diagonal block): keep q >= k,
            # i.e. f - p >= 0
            nc.gpsimd.memset(tmask[:, :, :], 1.0)
            nc.gpsimd.affine_select(
                out=tmask[:, :, :], in_=tmask[:, :, :],
                compare_op=mybir.AluOpType.is_ge, fill=0.0,
                base=0, pattern=[[0, 2], [1, 128]],
                channel_multiplier=-1,
            )
            # selector matrices for broadcasting softmax denominators across
            # partitions on the PE: sel[:, row, :].T @ r replicates r's row
            # `row` onto 64 output partitions
            nc.gpsimd.memset(sel[:, :, :], 1.0)
            nc.gpsimd.affine_select(
                out=sel[:, :, :], in_=sel[:, :, :],
                compare_op=mybir.AluOpType.is_equal, fill=0.0,
                base=0, pattern=[[-1, 16], [0, 64]],
                channel_multiplier=1,
            )

            # ---------- phases 2-4 fused: proj + attention per q-chunk ------
            def proj_chunk(c):
                # Q^T / K^T / V projections for chunk c's q range and k tiles
                cs = slice(c * 512, (c + 1) * 512)
                for dst, wsb in ((kth, wksb), (qth, wqsb)):
                    for pair in range(2):
                        pcols = slice(pair * 128, (pair + 1) * 128)
                        ps = psP.tile([128, 512], F32, tag="proj", name="ps_proj")
                        for k in range(NKT):
                            nc.tensor.matmul(ps[:, :], wsb[:, k, pcols], xT[:, k, cs],
                                             start=(k == 0), stop=(k == NKT - 1))
                        nc.vector.tensor_copy(out=dst[:, pair, cs], in_=ps[:, :])
                for mt in range(4 * c, 4 * c + 4):
                    ps = psP.tile([128, GCOLS], F32, tag="proj", name="ps_v")
                    for k in range(NKT):
                        nc.tensor.matmul(ps[:, :], xT[:, k, mt * 128:(mt + 1) * 128],
                                         wvsb[:, k, :],
                                         start=(k == 0), stop=(k == NKT - 1))
                    nc.vector.tensor_copy(
                        out=v1[:, mt, :, 0:DH],
                        in_=ps[:, :].rearrange("p (h d) -> p h d", h=HG),
                    )

            rdram = dramp.tile([64, 128], F32)

            def out_proj(c):
                # output projection (partial) for chunk c's sequence tiles
                for mt in range(4 * c, 4 * c + 4):
                    ms = slice(mt * 128, (mt + 1) * 128)
                    for nh in range(2):
                        ns = slice(nh * 512, (nh + 1) * 512)
                        ps = psP.tile([128, 512], F32, tag="proj", name="ps_out")
                        for p in range(2):
                            nc.tensor.matmul(ps[:, :], osb[:, p, ms], wosb[:, p, ns],
                                             start=(p == 0), stop=(p == 1))
                        ot = outs.tile([128, 512], F32, tag="ot", name="ot")
                        nc.vector.tensor_add(ot[:, :], ps[:, :], bosb[:, ns])
                        nc.sync.dma_start(out=out[ms, ns], in_=ot[:, :])

            for c in range(NQC):
                proj_chunk(c)
            for c in range(NQC):
                qs = slice(c * 512, (c + 1) * 512)
                nkt = 4 * (c + 1)
                lc = lp.tile([16, 128], F32, tag="lc", name="lc")
                for pair in range(2):
                    hA, hB = 2 * pair, 2 * pair + 1
                    poA = psO.tile([65, 512], F32, tag="oA", name="poA")
                    poB = psO.tile([65, 512], F32, tag="oB", name="poB")
                    for k in range(nkt):
                        ks = slice(k * 128, (k + 1) * 128)
                        j = k - 4 * c
                        # on diagonal blocks, q columns f < 128*j are fully
                        # masked: skip their S^T stream, exp, and P@V
                        # accumulation entirely
                        fs = 128 * max(j, 0)
                        qsj = slice(c * 512 + fs, (c + 1) * 512)
                        ss = psS.tile([128, 1024], F32, tag="s", name="ss")
                        # S^T = K^T.T @ Q^T for both heads of the pair
                        # (row-tiled: head A rows 0:64, head B rows 64:128)
                        nc.tensor.matmul(ss[:, fs:512], kth[0:64, pair, ks],
                                         qth[0:64, pair, qsj],
                                         start=True, stop=True)
                        nc.tensor.matmul(ss[:, 512 + fs:1024], kth[64:128, pair, ks],
                                         qth[64:128, pair, qsj],
                                         start=True, stop=True)
                        pt = ptp.tile([128, 2, 512], BF16, tag="pt", name="pt")
                        if j < 0:
                            # below the diagonal: everything unmasked
                            nc.scalar.activation(out=pt[:, :, :], in_=ss[:, :],
                                                 func=mybir.ActivationFunctionType.Exp,
                                                 scale=SCALE)
                        else:
                            # diagonal block: exp the live columns, then zero
                            # the triangular boundary sub-block's upper part
                            nc.scalar.activation(
                                out=pt[:, :, fs:], in_=ss[:, :].rearrange(
                                    "p (e f) -> p e f", e=2)[:, :, fs:],
                                func=mybir.ActivationFunctionType.Exp,
                                scale=SCALE)
                            nc.vector.tensor_mul(pt[:, :, fs:fs + 128],
                                                 pt[:, :, fs:fs + 128],
                                                 tmask[:, :, :])
                        nc.tensor.matmul(poA[:, fs:], v1[:, k, hA, :],
                                         pt[:, 0, fs:],
                                         start=(k == 0), stop=(k == nkt - 1))
                        nc.tensor.matmul(poB[:, fs:], v1[:, k, hB, :],
                                         pt[:, 1, fs:],
                                         start=(k == 0), stop=(k == nkt - 1))
                    nc.vector.tensor_copy(out=ost[:, hA, qs], in_=poA[:, :])
                    nc.vector.tensor_copy(out=ost[:, hB, qs], in_=poB[:, :])
                    # stash both heads' l rows (partition 64) into the packed
                    # per-chunk l tile, split into 4 q-subblocks of 128 per
                    # head so the reciprocal spreads over 16 DVE lanes
                    for h in (hA, hB):
                        nc.sync.dma_start(
                            out=lc[4 * h: 4 * h + 4, :],
                            in_=ost[64:65, h, qs],
                        )

                # softmax denominators for this chunk, then normalize O^T.
                # For chunks that still have attention work behind them the
                # reciprocal rows bounce through DRAM (partition-broadcast
                # DMA, zero PE cost, latency hidden by the next chunk); the
                # final chunk's chain is exposed, so it broadcasts via tiny
                # selector matmuls on the PE instead (lower latency, and the
                # PE work fills the tail bubble).
                last = c == NQC - 1
                rc32 = lp.tile([16, 128], F32, tag="rc32", name="rc32")
                nc.vector.reciprocal(out=rc32[:, :], in_=lc[:, :])
                if last:
                    rc = lp.tile([16, 128], BF16, tag="rc", name="rc")
                    nc.vector.tensor_copy(out=rc[:, :], in_=rc32[:, :])
                else:
                    nc.sync.dma_start(out=rdram[c * 16:(c + 1) * 16, :],
                                      in_=rc32[:, :])
                    rb = lp.tile([64, 16, 128], F32, tag="rb", name="rb")
                    src = rdram[c * 16:(c + 1) * 16, :]
                    bcast = bass.AP(tensor=src.tensor, offset=src.offset,
                                    ap=[[0, 64]] + list(src.ap))
                    nc.sync.dma_start(out=rb[:, :, :], in_=bcast)
                for h in range(HG):
                    p, e = divmod(h, 2)
                    if last:
                        rbp = psP.tile([64, 512], F32, tag="proj", name="rb_ps")
                        for s in range(4):
                            nc.tensor.matmul(rbp[:, s * 128:(s + 1) * 128],
                                             sel[:, 4 * h + s, :], rc[:, :],
                                             start=True, stop=True)
                        r_src = rbp[:, :].rearrange("p (s f) -> p s f", f=128)
                    else:
                        r_src = rb[:, 4 * h: 4 * h + 4, :]
                    o_src = ost[0:64, h, qs].rearrange("p (s f) -> p s f", f=128)
                    if e == 0:
                        nc.vector.tensor_mul(
                            osb[0:64, p, qs].rearrange("p (s f) -> p s f", f=128),
                            o_src, r_src)
                    else:
                        # odd head lives on partitions 64:128 of the pair
                        # tensor; DVE can't cross partitions, so stage + DMA
                        onst = outs.tile([64, 512], BF16, tag="onst", name="onst")
                        nc.vector.tensor_mul(
                            onst[:, :].rearrange("p (s f) -> p s f", f=128),
                            o_src, r_src)
                        nc.sync.dma_start(out=osb[64:128, p, qs], in_=onst[:, :])

                # out-proj lags one chunk behind so its PE work can fill the
                # normalize-chain bubble of the final chunk
                if c >= 1:
                    out_proj(c - 1)
            out_proj(NQC - 1)

    nc.compile()
    return nc


def _in_maps(inputs):
    bf = ml_dtypes.bfloat16
    x = np.asarray(inputs["x"], np.float32)
    Wq = np.asarray(inputs["Wq"], np.float32).astype(bf)
    Wkv = np.asarray(inputs["Wkv"], np.float32).astype(bf)
    Wo = np.asarray(inputs["Wo"], np.float32).astype(bf)
    bo = np.asarray(inputs["bo"], np.float32)
    maps = []
    for i in range(8):
        b, g = divmod(i, GROUPS)
        cs = slice(g * GCOLS, (g + 1) * GCOLS)
        maps.append(dict(
            xt=np.ascontiguousarray(x[b].T.astype(bf)),
            wq=np.ascontiguousarray(Wq[:, cs]),
            wk=np.ascontiguousarray(Wkv[:, cs]),
            wv=np.ascontiguousarray(Wkv[:, DIM + g * GCOLS: DIM + (g + 1) * GCOLS]),
            wo=np.ascontiguousarray(Wo[cs, :]),
            bo=np.ascontiguousarray(bo / GROUPS),
        ))
    return maps


_NC = None


def _get_nc():
    global _NC
    if _NC is None:
        nc = build()
        nc.finalize()
        _NC = nc
    return _NC


def run(inputs, trace=False, **kwargs):
    maps = _in_maps(inputs)
    res = run_bass_kernel_spmd(_get_nc(), maps, core_ids=list(range(8)),
                               trace=trace, **kwargs)
    out = np.empty((B, N, DIM), np.float32)
    for b in range(B):
        acc = res.results[4 * b]["out"].astype(np.float32).copy()
        for g in range(1, GROUPS):
            acc += res.results[4 * b + g]["out"]
        out[b] = acc
    return out, res


def kernel(**inputs):
    out, _ = run(inputs, trace=False)
    return out


# revision 30
# speedup vs baseline: 1.1905x; 1.0478x over previous
"""Trainium2 Bass kernel: causal multi-head attention (B=2, N=2048, DIM=1024, H=16, DH=64).

Sharding over 8 NeuronCores: data-parallel on batch (2) x tensor-parallel on
head groups (4 heads / core).  Each core computes Q/K/V projections for its 4
heads, causal flash-style attention, and a partial output projection against
its slice of Wo.  The 4 partial outputs per batch are summed to form the full
output.

Layout notes (per core):
  - x arrives pre-transposed and pre-cast from the host as xt = bf16(x[b].T)
    (DIM, N) so the contraction dim of every projection matmul sits on SBUF
    partitions and the load is half the bytes.
  - Q^T / K^T are kept with head-dim on partitions: pair tensors (128, 2, N)
    where partitions 0:64 hold head 2p and 64:128 hold head 2p+1.  The two
    heads of a pair issue row-tiled (tile_position) matmuls that can run
    concurrently on the PE array (K=64 each).
  - Scores are computed transposed: S^T (k_seq on partitions, q on free), so
    softmax needs no max subtraction (scores ~ N(0,1)) and P^T feeds the
    P@V matmul directly with K=128.  Row sums l come for free from a ones
    column appended to V (lhsT = [V | 1], out rows 0:64 = O^T, row 64 = l).
  - The q-chunk loop is outermost so that softmax normalization and the
    output projection of chunk c pipeline with the attention of chunk c+1
    (avoids a serialized tail that lets the PE HAM clock-gate go cold).
"""

import numpy as np
import ml_dtypes

import concourse.bass as bass
import concourse.bacc as bacc
import concourse.tile as tile
from concourse import mybir
from concourse.bass_utils import run_bass_kernel_spmd

B, N, DIM, H, DH = 2, 2048, 1024, 16, 64
HG = 4                  # heads per core
GROUPS = 4              # tensor-parallel degree (head groups)
GCOLS = HG * DH         # 256 inner columns per core
NKT = DIM // 128        # 8 contraction tiles for projections
NQC = N // 512          # 4 query chunks
NMT = N // 128          # 16 sequence tiles
SCALE = DH ** -0.5

F32 = mybir.dt.float32
BF16 = mybir.dt.bfloat16


def build():
    nc = bacc.Bacc("TRN2", target_bir_lowering=False, debug=True)

    xt = nc.declare_dram_parameter("xt", [DIM, N], BF16, isOutput=False)
    wq = nc.declare_dram_parameter("wq", [DIM, GCOLS], BF16, isOutput=False)
    wk = nc.declare_dram_parameter("wk", [DIM, GCOLS], BF16, isOutput=False)
    wv = nc.declare_dram_parameter("wv", [DIM, GCOLS], BF16, isOutput=False)
    wo = nc.declare_dram_parameter("wo", [GCOLS, DIM], BF16, isOutput=False)
    bo = nc.declare_dram_parameter("bo", [DIM], F32, isOutput=False)
    out = nc.declare_dram_parameter("out", [N, DIM], F32, isOutput=True)

    with tile.TileContext(nc) as tc:
        with (
            tc.tile_pool(name="const", bufs=1) as const,
            tc.tile_pool(name="ptp", bufs=3) as ptp,
            tc.tile_pool(name="lp", bufs=2) as lp,
            tc.tile_pool(name="outs", bufs=3) as outs,
            tc.tile_pool(name="psS", bufs=2, space="PSUM") as psS,
            tc.tile_pool(name="psO", bufs=1, space="PSUM") as psO,
            tc.tile_pool(name="psP", bufs=2, space="PSUM") as psP,
            tc.tile_pool(name="dramp", bufs=1, space="DRAM") as dramp,
        ):
            # ---------------- persistent tiles ----------------
            xT = const.tile([128, NKT, N], BF16)          # x^T (dim on partitions)
            wqsb = const.tile([128, NKT, GCOLS], BF16)
            wksb = const.tile([128, NKT, GCOLS], BF16)
            wvsb = const.tile([128, NKT, GCOLS], BF16)
            wosb = const.tile([128, 2, DIM], BF16)        # Wo rows, head-pair layout
            bosb = const.tile([128, DIM], F32)            # bo broadcast to 128 parts
            tmask = const.tile([128, 2, 128], BF16)       # triangular binary mask
            v1 = const.tile([128, NMT, HG, DH + 1], BF16)  # [V | ones]
            qth = const.tile([128, 2, N], BF16)           # Q^T head pairs
            kth = const.tile([128, 2, N], BF16)           # K^T head pairs
            ost = const.tile([65, HG, N], F32)            # unnormalized O^T + l row
            sel = const.tile([16, 16, 64], BF16)          # PE-broadcast selectors
            osb = const.tile([128, 2, N], BF16)           # normalized O^T, pair layout

            # ---------------- phase 1: loads (HWDGE, already bf16) ----------
            # wk first (the first projection emitted is K^T), then x per
            # k-tile so the K^T matmuls can start as soon as tiles land.
            nc.sync.dma_start(out=wksb[:, :, :],
                              in_=wk[:, :].rearrange("(t p) n -> p t n", p=128))
            for k in range(NKT):
                nc.sync.dma_start(out=xT[:, k, :], in_=xt[k * 128:(k + 1) * 128, :])
            for wsb, wdr in ((wqsb, wq), (wvsb, wv)):
                nc.sync.dma_start(
                    out=wsb[:, :, :],
                    in_=wdr[:, :].rearrange("(t p) n -> p t n", p=128),
                )
            for h in range(HG):
                p, e = divmod(h, 2)
                nc.sync.dma_start(out=wosb[e * 64:(e + 1) * 64, p, :],
                                  in_=wo[h * DH:(h + 1) * DH, :])
            bo_ap = bo[:]
            bo_bcast = bass.AP(tensor=bo_ap.tensor, offset=bo_ap.offset,
                               ap=[[0, 128]] + list(bo_ap.ap))
            nc.sync.dma_start(out=bosb[:, :], in_=bo_bcast)

            # ones column of [V | 1]
            nc.vector.memset(v1[:, :, :, DH:DH + 1], 1.0)
            # triangular binary mask for the 128-wide diagonal boundary
            # sub-block (identical for every diagonal block): keep q >= k,
            # i.e. f - p >= 0
            nc.gpsimd.memset(tmask[:, :, :], 1.0)
            nc.gpsimd.affine_select(
                out=tmask[:, :, :], in_=tmask[:, :, :],
                compare_op=mybir.AluOpType.is_ge, fill=0.0,
                base=0, pattern=[[0, 2], [1, 128]],
                channel_multiplier=-1,
            )
            # selector matrices for broadcasting softmax denominators across
            # partitions on the PE: sel[:, row, :].T @ r replicates r's row
            # `row` onto 64 output partitions
            nc.gpsimd.memset(sel[:, :, :], 1.0)
            nc.gpsimd.affine_select(
                out=sel[:, :, :], in_=sel[:, :, :],
                compare_op=mybir.AluOpType.is_equal, fill=0.0,
                base=0, pattern=[[-1, 16], [0, 64]],
                channel_multiplier=1,
            )

            # ---------- phases 2-4 fused: proj + attention per q-chunk ------
            def proj_chunk(c):
                # Q^T / K^T / V projections for chunk c's q range and k tiles
                cs = slice(c * 512, (c + 1) * 512)
                for dst, wsb in ((kth, wksb), (qth, wqsb)):
                    for pair in range(2):
                        pcols = slice(pair * 128, (pair + 1) * 128)
                        ps = psP.tile([128, 512], F32, tag="proj", name="ps_proj")
                        for k in range(NKT):
                            nc.tensor.matmul(ps[:, :], wsb[:, k, pcols], xT[:, k, cs],
                                             start=(k == 0), stop=(k == NKT - 1))
                        nc.vector.tensor_copy(out=dst[:, pair, cs], in_=ps[:, :])
                for mt in range(4 * c, 4 * c + 4):
                    ps = psP.tile([128, GCOLS], F32, tag="proj", name="ps_v")
                    for k in range(NKT):
                        nc.tensor.matmul(ps[:, :], xT[:, k, mt * 128:(mt + 1) * 128],
                                         wvsb[:, k, :],
                                         start=(k == 0), stop=(k == NKT - 1))
                    nc.vector.tensor_copy(
                        out=v1[:, mt, :, 0:DH],
                        in_=ps[:, :].rearrange("p (h d) -> p h d", h=HG),
                    )

            rdram = dramp.tile([64, 128], F32)

            def out_proj(c):
                # output projection (partial) for chunk c's sequence tiles
                for mt in range(4 * c, 4 * c + 4):
                    ms = slice(mt * 128, (mt + 1) * 128)
                    for nh in range(2):
                        ns = slice(nh * 512, (nh + 1) * 512)
                        ps = psP.tile([128, 512], F32, tag="proj", name="ps_out")
                        for p in range(2):
                            nc.tensor.matmul(ps[:, :], osb[:, p, ms], wosb[:, p, ns],
                                             start=(p == 0), stop=(p == 1))
                        ot = outs.tile([128, 512], F32, tag="ot", name="ot")
                        nc.vector.tensor_add(ot[:, :], ps[:, :], bosb[:, ns])
                        nc.sync.dma_start(out=out[ms, ns], in_=ot[:, :])

            proj_chunk(0)
            attn_chunks = []

            def emit_rest_proj():
                for cc in range(1, NQC):
                    proj_chunk(cc)

            for c in range(NQC):
                if c == 1:
                    emit_rest_proj()
                qs = slice(c * 512, (c + 1) * 512)
                nkt = 4 * (c + 1)
                lc = lp.tile([16, 128], F32, tag="lc", name="lc")
                for pair in range(2):
                    hA, hB = 2 * pair, 2 * pair + 1
                    poA = psO.tile([65, 512], F32, tag="oA", name="poA")
                    poB = psO.tile([65, 512], F32, tag="oB", name="poB")
                    for k in range(nkt):
                        ks = slice(k * 128, (k + 1) * 128)
                        j = k - 4 * c
                        # on diagonal blocks, q columns f < 128*j are fully
                        # masked: skip their S^T stream, exp, and P@V
                        # accumulation entirely
                        fs = 128 * max(j, 0)
                        qsj = slice(c * 512 + fs, (c + 1) * 512)
                        ss = psS.tile([128, 1024], F32, tag="s", name="ss")
                        # S^T = K^T.T @ Q^T for both heads of the pair
                        # (row-tiled: head A rows 0:64, head B rows 64:128)
                        nc.tensor.matmul(ss[:, fs:512], kth[0:64, pair, ks],
                                         qth[0:64, pair, qsj],
                                         start=True, stop=True)
                        nc.tensor.matmul(ss[:, 512 + fs:1024], kth[64:128, pair, ks],
                                         qth[64:128, pair, qsj],
                                         start=True, stop=True)
                        pt = ptp.tile([128, 2, 512], BF16, tag="pt", name="pt")
                        if j < 0:
                            # below the diagonal: everything unmasked
                            nc.scalar.activation(out=pt[:, :, :], in_=ss[:, :],
                                                 func=mybir.ActivationFunctionType.Exp,
                                                 scale=SCALE)
                        else:
                            # diagonal block: exp the live columns, then zero
                            # the triangular boundary sub-block's upper part
                            nc.scalar.activation(
                                out=pt[:, :, fs:], in_=ss[:, :].rearrange(
                                    "p (e f) -> p e f", e=2)[:, :, fs:],
                                func=mybir.ActivationFunctionType.Exp,
                                scale=SCALE)
                            nc.vector.tensor_mul(pt[:, :, fs:fs + 128],
                                                 pt[:, :, fs:fs + 128],
                                                 tmask[:, :, :])
                        nc.tensor.matmul(poA[:, fs:], v1[:, k, hA, :],
                                         pt[:, 0, fs:],
                                         start=(k == 0), stop=(k == nkt - 1))
                        nc.tensor.matmul(poB[:, fs:], v1[:, k, hB, :],
                                         pt[:, 1, fs:],
                                         start=(k == 0), stop=(k == nkt - 1))
                    nc.vector.tensor_copy(out=ost[:, hA, qs], in_=poA[:, :])
                    nc.vector.tensor_copy(out=ost[:, hB, qs], in_=poB[:, :])
                    # stash both heads' l rows (partition 64) into the packed
                    # per-chunk l tile, split into 4 q-subblocks of 128 per
                    # head so the reciprocal spreads over 16 DVE lanes
                    for h in (hA, hB):
                        nc.sync.dma_start(
                            out=lc[4 * h: 4 * h + 4, :],
                            in_=ost[64:65, h, qs],
                        )

                # softmax denominators for this chunk, then normalize O^T.
                # For chunks that still have attention work behind them the
                # reciprocal rows bounce through DRAM (partition-broadcast
                # DMA, zero PE cost, latency hidden by the next chunk); the
                # final chunk's chain is exposed, so it broadcasts via tiny
                # selector matmuls on the PE instead (lower latency, and the
                # PE work fills the tail bubble).
                last = c == NQC - 1
                rc32 = lp.tile([16, 128], F32, tag="rc32", name="rc32")
                nc.vector.reciprocal(out=rc32[:, :], in_=lc[:, :])
                if last:
                    rc = lp.tile([16, 128], BF16, tag="rc", name="rc")
                    nc.vector.tensor_copy(out=rc[:, :], in_=rc32[:, :])
                else:
                    nc.sync.dma_start(out=rdram[c * 16:(c + 1) * 16, :],
                                      in_=rc32[:, :])
                    rb = lp.tile([64, 16, 128], F32, tag="rb", name="rb")
                    src = rdram[c * 16:(c + 1) * 16, :]
                    bcast = bass.AP(tensor=src.tensor, offset=src.offset,
                                    ap=[[0, 64]] + list(src.ap))
                    nc.sync.dma_start(out=rb[:, :, :], in_=bcast)
                for h in range(HG):
                    p, e = divmod(h, 2)
                    if last:
                        rbp = psP.tile([64, 512], F32, tag="proj", name="rb_ps")
                        for s in range(4):
                            nc.tensor.matmul(rbp[:, s * 128:(s + 1) * 128],
                                             sel[:, 4 * h + s, :], rc[:, :],
                                             start=True, stop=True)
                        r_src = rbp[:, :].rearrange("p (s f) -> p s f", f=128)
                    else:
                        r_src = rb[:, 4 * h: 4 * h + 4, :]
                    o_src = ost[0:64, h, qs].rearrange("p (s f) -> p s f", f=128)
                    if e == 0:
                        nc.vector.tensor_mul(
                            osb[0:64, p, qs].rearrange("p (s f) -> p s f", f=128),
                            o_src, r_src)
                    else:
                        # odd head lives on partitions 64:128 of the pair
                        # tensor; DVE can't cross partitions, so stage + DMA
                        onst = outs.tile([64, 512], BF16, tag="onst", name="onst")
                        nc.vector.tensor_mul(
                            onst[:, :].rearrange("p (s f) -> p s f", f=128),
                            o_src, r_src)
                        nc.sync.dma_start(out=osb[64:128, p, qs], in_=onst[:, :])

                # out-proj lags one chunk behind so its PE work can fill the
                # normalize-chain bubble of the final chunk
                if c >= 1:
                    out_proj(c - 1)
            out_proj(NQC - 1)

    nc.compile()
    return nc


def _in_maps(inputs):
    bf = ml_dtypes.bfloat16
    x = np.asarray(inputs["x"], np.float32)
    Wq = np.asarray(inputs["Wq"], np.float32).astype(bf)
    Wkv = np.asarray(inputs["Wkv"], np.float32).astype(bf)
    Wo = np.asarray(inputs["Wo"], np.float32).astype(bf)
    bo = np.asarray(inputs["bo"], np.float32)
    maps = []
    for i in range(8):
        b, g = divmod(i, GROUPS)
        cs = slice(g * GCOLS, (g + 1) * GCOLS)
        maps.append(dict(
            xt=np.ascontiguousarray(x[b].T.astype(bf)),
            wq=np.ascontiguousarray(Wq[:, cs]),
            wk=np.ascontiguousarray(Wkv[:, cs]),
            wv=np.ascontiguousarray(Wkv[:, DIM + g * GCOLS: DIM + (g + 1) * GCOLS]),
            wo=np.ascontiguousarray(Wo[cs, :]),
            bo=np.ascontiguousarray(bo / GROUPS),
        ))
    return maps


_NC = None


def _get_nc():
    global _NC
    if _NC is None:
        nc = build()
        nc.finalize()
        _NC = nc
    return _NC


def run(inputs, trace=False, **kwargs):
    maps = _in_maps(inputs)
    res = run_bass_kernel_spmd(_get_nc(), maps, core_ids=list(range(8)),
                               trace=trace, **kwargs)
    out = np.empty((B, N, DIM), np.float32)
    for b in range(B):
        acc = res.results[4 * b]["out"].astype(np.float32).copy()
        for g in range(1, GROUPS):
            acc += res.results[4 * b + g]["out"]
        out[b] = acc
    return out, res


def kernel(**inputs):
    out, _ = run(inputs, trace=False)
    return out


# revision 32
# speedup vs baseline: 1.1944x; 1.0033x over previous
"""Trainium2 Bass kernel: causal multi-head attention (B=2, N=2048, DIM=1024, H=16, DH=64).

Sharding over 8 NeuronCores: data-parallel on batch (2) x tensor-parallel on
head groups (4 heads / core).  Each core computes Q/K/V projections for its 4
heads, causal flash-style attention, and a partial output projection against
its slice of Wo.  The 4 partial outputs per batch are summed to form the full
output.

Layout notes (per core):
  - x arrives pre-transposed and pre-cast from the host as xt = bf16(x[b].T)
    (DIM, N) so the contraction dim of every projection matmul sits on SBUF
    partitions and the load is half the bytes.
  - Q^T / K^T are kept with head-dim on partitions: pair tensors (128, 2, N)
    where partitions 0:64 hold head 2p and 64:128 hold head 2p+1.  The two
    heads of a pair issue row-tiled (tile_position) matmuls that can run
    concurrently on the PE array (K=64 each).
  - Scores are computed transposed: S^T (k_seq on partitions, q on free), so
    softmax needs no max subtraction (scores ~ N(0,1)) and P^T feeds the
    P@V matmul directly with K=128.  Row sums l come for free from a ones
    column appended to V (lhsT = [V | 1], out rows 0:64 = O^T, row 64 = l).
  - The q-chunk loop is outermost so that softmax normalization and the
    output projection of chunk c pipeline with the attention of chunk c+1
    (avoids a serialized tail that lets the PE HAM clock-gate go cold).
"""

import numpy as np
import ml_dtypes

import concourse.bass as bass
import concourse.bacc as bacc
import concourse.tile as tile
from concourse import mybir
from concourse.bass_utils import run_bass_kernel_spmd

B, N, DIM, H, DH = 2, 2048, 1024, 16, 64
HG = 4                  # heads per core
GROUPS = 4              # tensor-parallel degree (head groups)
GCOLS = HG * DH         # 256 inner columns per core
NKT = DIM // 128        # 8 contraction tiles for projections
NQC = N // 512          # 4 query chunks
NMT = N // 128          # 16 sequence tiles
SCALE = DH ** -0.5

F32 = mybir.dt.float32
BF16 = mybir.dt.bfloat16


def build():
    nc = bacc.Bacc("TRN2", target_bir_lowering=False, debug=True)

    xt = nc.declare_dram_parameter("xt", [DIM, N], BF16, isOutput=False)
    wq = nc.declare_dram_parameter("wq", [DIM, GCOLS], BF16, isOutput=False)
    wk = nc.declare_dram_parameter("wk", [DIM, GCOLS], BF16, isOutput=False)
    wv = nc.declare_dram_parameter("wv", [DIM, GCOLS], BF16, isOutput=False)
    wo = nc.declare_dram_parameter("wo", [GCOLS, DIM], BF16, isOutput=False)
    bo = nc.declare_dram_parameter("bo", [DIM], F32, isOutput=False)
    out = nc.declare_dram_parameter("out", [N, DIM], F32, isOutput=True)

    with tile.TileContext(nc) as tc:
        with (
            tc.tile_pool(name="const", bufs=1) as const,
            tc.tile_pool(name="ptp", bufs=3) as ptp,
            tc.tile_pool(name="lp", bufs=2) as lp,
            tc.tile_pool(name="outs", bufs=3) as outs,
            tc.tile_pool(name="psS", bufs=2, space="PSUM") as psS,
            tc.tile_pool(name="psO", bufs=1, space="PSUM") as psO,
            tc.tile_pool(name="psP", bufs=2, space="PSUM") as psP,
            tc.tile_pool(name="dramp", bufs=1, space="DRAM") as dramp,
        ):
            # ---------------- persistent tiles ----------------
            xT = const.tile([128, NKT, N], BF16)          # x^T (dim on partitions)
            wqsb = const.tile([128, NKT, GCOLS], BF16)
            wksb = const.tile([128, NKT, GCOLS], BF16)
            wvsb = const.tile([128, NKT, GCOLS], BF16)
            wosb = const.tile([128, 2, DIM], BF16)        # Wo rows, head-pair layout
            bosb = const.tile([128, DIM], F32)            # bo broadcast to 128 parts
            tmask = const.tile([128, 2, 128], BF16)       # triangular binary mask
            v1 = const.tile([128, NMT, HG, DH + 1], BF16)  # [V | ones]
            qth = const.tile([128, 2, N], BF16)           # Q^T head pairs
            kth = const.tile([128, 2, N], BF16)           # K^T head pairs
            ost = const.tile([65, HG, N], F32)            # unnormalized O^T + l row
            sel = const.tile([16, 16, 64], BF16)          # PE-broadcast selectors
            osb = const.tile([128, 2, N], BF16)           # normalized O^T, pair layout

            # ---------------- phase 1: loads (HWDGE, already bf16) ----------
            # wk first (the first projection emitted is K^T), then x per
            # k-tile so the K^T matmuls can start as soon as tiles land.
            nc.sync.dma_start(out=wksb[:, :, :],
                              in_=wk[:, :].rearrange("(t p) n -> p t n", p=128))
            for k in range(NKT):
                nc.sync.dma_start(out=xT[:, k, :], in_=xt[k * 128:(k + 1) * 128, :])
            for wsb, wdr in ((wqsb, wq), (wvsb, wv)):
                nc.sync.dma_start(
                    out=wsb[:, :, :],
                    in_=wdr[:, :].rearrange("(t p) n -> p t n", p=128),
                )
            for h in range(HG):
                p, e = divmod(h, 2)
                nc.sync.dma_start(out=wosb[e * 64:(e + 1) * 64, p, :],
                                  in_=wo[h * DH:(h + 1) * DH, :])
            bo_ap = bo[:]
            bo_bcast = bass.AP(tensor=bo_ap.tensor, offset=bo_ap.offset,
                               ap=[[0, 128]] + list(bo_ap.ap))
            nc.sync.dma_start(out=bosb[:, :], in_=bo_bcast)

            # ones column of [V | 1]
            nc.vector.memset(v1[:, :, :, DH:DH + 1], 1.0)
            # triangular binary mask for the 128-wide diagonal boundary
            # sub-block (identical for every diagonal block): keep q >= k,
            # i.e. f - p >= 0
            nc.gpsimd.memset(tmask[:, :, :], 1.0)
            nc.gpsimd.affine_select(
                out=tmask[:, :, :], in_=tmask[:, :, :],
                compare_op=mybir.AluOpType.is_ge, fill=0.0,
                base=0, pattern=[[0, 2], [1, 128]],
                channel_multiplier=-1,
            )
            # selector matrices for broadcasting softmax denominators across
            # partitions on the PE: sel[:, row, :].T @ r replicates r's row
            # `row` onto 64 output partitions
            nc.gpsimd.memset(sel[:, :, :], 1.0)
            nc.gpsimd.affine_select(
                out=sel[:, :, :], in_=sel[:, :, :],
                compare_op=mybir.AluOpType.is_equal, fill=0.0,
                base=0, pattern=[[-1, 16], [0, 64]],
                channel_multiplier=1,
            )

            # ---------- phases 2-4 fused: proj + attention per q-chunk ------
            def proj_chunk(c):
                # Q^T / K^T / V projections for chunk c's q range and k tiles
                cs = slice(c * 512, (c + 1) * 512)
                for dst, wsb in ((kth, wksb), (qth, wqsb)):
                    for pair in range(2):
                        pcols = slice(pair * 128, (pair + 1) * 128)
                        ps = psP.tile([128, 512], F32, tag="proj", name="ps_proj")
                        for k in range(NKT):
                            nc.tensor.matmul(ps[:, :], wsb[:, k, pcols], xT[:, k, cs],
                                             start=(k == 0), stop=(k == NKT - 1))
                        nc.vector.tensor_copy(out=dst[:, pair, cs], in_=ps[:, :])
                for mt in range(4 * c, 4 * c + 4):
                    ps = psP.tile([128, GCOLS], F32, tag="proj", name="ps_v")
                    for k in range(NKT):
                        nc.tensor.matmul(ps[:, :], xT[:, k, mt * 128:(mt + 1) * 128],
                                         wvsb[:, k, :],
                                         start=(k == 0), stop=(k == NKT - 1))
                    nc.vector.tensor_copy(
                        out=v1[:, mt, :, 0:DH],
                        in_=ps[:, :].rearrange("p (h d) -> p h d", h=HG),
                    )

            rdram = dramp.tile([64, 128], F32)

            def out_proj(c):
                # output projection (partial) for chunk c's sequence tiles
                for mt in range(4 * c, 4 * c + 4):
                    ms = slice(mt * 128, (mt + 1) * 128)
                    for nh in range(2):
                        ns = slice(nh * 512, (nh + 1) * 512)
                        ps = psP.tile([128, 512], F32, tag="proj", name="ps_out")
                        for p in range(2):
                            nc.tensor.matmul(ps[:, :], osb[:, p, ms], wosb[:, p, ns],
                                             start=(p == 0), stop=(p == 1))
                        ot = outs.tile([128, 512], F32, tag="ot", name="ot")
                        nc.vector.tensor_add(ot[:, :], ps[:, :], bosb[:, ns])
                        nc.sync.dma_start(out=out[ms, ns], in_=ot[:, :])

            proj_chunk(0)
            attn_chunks = []

            def emit_rest_proj():
                for cc in range(1, NQC):
                    proj_chunk(cc)

            for c in range(NQC):
                if c == 1:
                    emit_rest_proj()
                qs = slice(c * 512, (c + 1) * 512)
                nkt = 4 * (c + 1)
                lc = lp.tile([16, 128], F32, tag="lc", name="lc")
                for pair in range(2):
                    hA, hB = 2 * pair, 2 * pair + 1
                    poA = psO.tile([65, 512], F32, tag="oA", name="poA")
                    poB = psO.tile([65, 512], F32, tag="oB", name="poB")
                    for k in range(nkt):
                        ks = slice(k * 128, (k + 1) * 128)
                        j = k - 4 * c
                        # on diagonal blocks, q columns f < 128*j are fully
                        # masked: skip their S^T stream, exp, and P@V
                        # accumulation entirely
                        fs = 128 * max(j, 0)
                        qsj = slice(c * 512 + fs, (c + 1) * 512)
                        ss = psS.tile([128, 1024], F32, tag="s", name="ss")
                        # S^T = K^T.T @ Q^T for both heads of the pair
                        # (row-tiled: head A rows 0:64, head B rows 64:128)
                        nc.tensor.matmul(ss[:, fs:512], kth[0:64, pair, ks],
                                         qth[0:64, pair, qsj],
                                         start=True, stop=True)
                        nc.tensor.matmul(ss[:, 512 + fs:1024], kth[64:128, pair, ks],
                                         qth[64:128, pair, qsj],
                                         start=True, stop=True)
                        pt = ptp.tile([128, 2, 512], BF16, tag="pt", name="pt")
                        if j < 0:
                            # below the diagonal: everything unmasked
                            nc.scalar.activation(out=pt[:, :, :], in_=ss[:, :],
                                                 func=mybir.ActivationFunctionType.Exp,
                                                 scale=SCALE)
                        else:
                            # diagonal block: exp the live columns, then zero
                            # the triangular boundary sub-block's upper part
                            nc.scalar.activation(
                                out=pt[:, :, fs:], in_=ss[:, :].rearrange(
                                    "p (e f) -> p e f", e=2)[:, :, fs:],
                                func=mybir.ActivationFunctionType.Exp,
                                scale=SCALE)
                            nc.vector.tensor_mul(pt[:, :, fs:fs + 128],
                                                 pt[:, :, fs:fs + 128],
                                                 tmask[:, :, :])
                        nc.tensor.matmul(poA[:, fs:], v1[:, k, hA, :],
                                         pt[:, 0, fs:],
                                         start=(k == 0), stop=(k == nkt - 1))
                        nc.tensor.matmul(poB[:, fs:], v1[:, k, hB, :],
                                         pt[:, 1, fs:],
                                         start=(k == 0), stop=(k == nkt - 1))
                    nc.vector.tensor_copy(out=ost[:, hA, qs], in_=poA[:, :])
                    nc.vector.tensor_copy(out=ost[:, hB, qs], in_=poB[:, :])
                    # stash both heads' l rows (partition 64) into the packed
                    # per-chunk l tile, split into 4 q-subblocks of 128 per
                    # head so the reciprocal spreads over 16 DVE lanes
                    for h in (hA, hB):
                        nc.sync.dma_start(
                            out=lc[4 * h: 4 * h + 4, :],
                            in_=ost[64:65, h, qs],
                        )

                # softmax denominators for this chunk, then normalize O^T.
                # For chunks that still have attention work behind them the
                # reciprocal rows bounce through DRAM (partition-broadcast
                # DMA, zero PE cost, latency hidden by the next chunk); the
                # final chunk's chain is exposed, so it broadcasts via tiny
                # selector matmuls on the PE instead (lower latency, and the
                # PE work fills the tail bubble).
                last = c == NQC - 1
                rc32 = lp.tile([16, 128], F32, tag="rc32", name="rc32")
                nc.vector.reciprocal(out=rc32[:, :], in_=lc[:, :])
                if last:
                    rc = lp.tile([16, 128], BF16, tag="rc", name="rc")
                    nc.vector.tensor_copy(out=rc[:, :], in_=rc32[:, :])
                else:
                    nc.sync.dma_start(out=rdram[c * 16:(c + 1) * 16, :],
                                      in_=rc32[:, :])
                    rb = lp.tile([64, 16, 128], F32, tag="rb", name="rb")
                    src = rdram[c * 16:(c + 1) * 16, :]
                    bcast = bass.AP(tensor=src.tensor, offset=src.offset,
                                    ap=[[0, 64]] + list(src.ap))
                    nc.sync.dma_start(out=rb[:, :, :], in_=bcast)
                for h in range(HG):
                    p, e = divmod(h, 2)
                    if last:
                        rbp = psP.tile([64, 512], F32, tag="proj", name="rb_ps")
                        for s in range(4):
                            nc.tensor.matmul(rbp[:, s * 128:(s + 1) * 128],
                                             sel[:, 4 * h + s, :], rc[:, :],
                                             start=True, stop=True)
                        r_src = rbp[:, :].rearrange("p (s f) -> p s f", f=128)
                    else:
                        r_src = rb[:, 4 * h: 4 * h + 4, :]
                    o_src = ost[0:64, h, qs].rearrange("p (s f) -> p s f", f=128)
                    if e == 0:
                        nc.vector.tensor_mul(
                            osb[0:64, p, qs].rearrange("p (s f) -> p s f", f=128),
                            o_src, r_src)
                    else:
                        # odd head lives on partitions 64:128 of the pair
                        # tensor; DVE can't cross partitions, so stage + DMA
                        onst = outs.tile([64, 512], BF16, tag="onst", name="onst")
                        nc.vector.tensor_mul(
                            onst[:, :].rearrange("p (s f) -> p s f", f=128),
                            o_src, r_src)
                        nc.sync.dma_start(out=osb[64:128, p, qs], in_=onst[:, :])

                # out-proj lags one chunk behind so its PE work can fill the
                # normalize-chain bubble of the final chunk
                if c >= 1:
                    out_proj(c - 1)
            out_proj(NQC - 1)

    nc.compile()
    return nc


def _in_maps(inputs):
    bf = ml_dtypes.bfloat16
    x = np.asarray(inputs["x"], np.float32)
    Wq = np.asarray(inputs["Wq"], np.float32).astype(bf)
    Wkv = np.asarray(inputs["Wkv"], np.float32).astype(bf)
    Wo = np.asarray(inputs["Wo"], np.float32).astype(bf)
    bo = np.asarray(inputs["bo"], np.float32)
    maps = []
    for i in range(8):
        b, g = divmod(i, GROUPS)
        cs = slice(g * GCOLS, (g + 1) * GCOLS)
        maps.append(dict(
            xt=np.ascontiguousarray(x[b].T.astype(bf)),
            wq=np.ascontiguousarray(Wq[:, cs]),
            wk=np.ascontiguousarray(Wkv[:, cs]),
            wv=np.ascontiguousarray(Wkv[:, DIM + g * GCOLS: DIM + (g + 1) * GCOLS]),
            wo=np.ascontiguousarray(Wo[cs, :]),
            bo=np.ascontiguousarray(bo / GROUPS),
        ))
    return maps


_NC = None


def _get_nc():
    global _NC
    if _NC is None:
        nc = build()
        nc.finalize()
        _NC = nc
    return _NC


def run(inputs, trace=False, **kwargs):
    maps = _in_maps(inputs)
    res = run_bass_kernel_spmd(_get_nc(), maps, core_ids=list(range(8)),
                               trace=trace, **kwargs)
    out = np.empty((B, N, DIM), np.float32)
    for b in range(B):
        acc = res.results[4 * b]["out"].astype(np.float32).copy()
        for g in range(1, GROUPS):
            acc += res.results[4 * b + g]["out"]
        out[b] = acc
    return out, res


def kernel(**inputs):
    out, _ = run(inputs, trace=False)
    return out


# revision 35
# speedup vs baseline: 1.2066x; 1.0102x over previous
"""Trainium2 Bass kernel: causal multi-head attention (B=2, N=2048, DIM=1024, H=16, DH=64).

Sharding over 8 NeuronCores: data-parallel on batch (2) x tensor-parallel on
head groups (4 heads / core).  Each core computes Q/K/V projections for its 4
heads, causal flash-style attention, and a partial output projection against
its slice of Wo.  The 4 partial outputs per batch are summed to form the full
output.

Layout notes (per core):
  - x arrives pre-transposed and pre-cast from the host as xt = bf16(x[b].T)
    (DIM, N) so the contraction dim of every projection matmul sits on SBUF
    partitions and the load is half the bytes.
  - Q^T / K^T are kept with head-dim on partitions: pair tensors (128, 2, N)
    where partitions 0:64 hold head 2p and 64:128 hold head 2p+1.  The two
    heads of a pair issue row-tiled (tile_position) matmuls that can run
    concurrently on the PE array (K=64 each).
  - Scores are computed transposed: S^T (k_seq on partitions, q on free), so
    softmax needs no max subtraction (scores ~ N(0,1)) and P^T feeds the
    P@V matmul directly with K=128.  Row sums l come for free from a ones
    column appended to V (lhsT = [V | 1], out rows 0:64 = O^T, row 64 = l).
  - The q-chunk loop is outermost so that softmax normalization and the
    output projection of chunk c pipeline with the attention of chunk c+1
    (avoids a serialized tail that lets the PE HAM clock-gate go cold).
"""

import numpy as np
import ml_dtypes

import concourse.bass as bass
import concourse.bacc as bacc
import concourse.tile as tile
from concourse import mybir
from concourse.bass_utils import run_bass_kernel_spmd

B, N, DIM, H, DH = 2, 2048, 1024, 16, 64
HG = 4                  # heads per core
GROUPS = 4              # tensor-parallel degree (head groups)
GCOLS = HG * DH         # 256 inner columns per core
NKT = DIM // 128        # 8 contraction tiles for projections
NQC = N // 512          # 4 query chunks
NMT = N // 128          # 16 sequence tiles
SCALE = DH ** -0.5

F32 = mybir.dt.float32
BF16 = mybir.dt.bfloat16


def build():
    nc = bacc.Bacc("TRN2", target_bir_lowering=False, debug=True)

    xt = nc.declare_dram_parameter("xt", [DIM, N], BF16, isOutput=False)
    wq = nc.declare_dram_parameter("wq", [DIM, GCOLS], BF16, isOutput=False)
    wk = nc.declare_dram_parameter("wk", [DIM, GCOLS], BF16, isOutput=False)
    wv = nc.declare_dram_parameter("wv", [DIM, GCOLS], BF16, isOutput=False)
    wo = nc.declare_dram_parameter("wo", [GCOLS, DIM], BF16, isOutput=False)
    bo = nc.declare_dram_parameter("bo", [DIM], F32, isOutput=False)
    out = nc.declare_dram_parameter("out", [N, DIM], BF16, isOutput=True)

    with tile.TileContext(nc) as tc:
        with (
            tc.tile_pool(name="const", bufs=1) as const,
            tc.tile_pool(name="ptp", bufs=3) as ptp,
            tc.tile_pool(name="lp", bufs=2) as lp,
            tc.tile_pool(name="outs", bufs=3) as outs,
            tc.tile_pool(name="psS", bufs=2, space="PSUM") as psS,
            tc.tile_pool(name="psO", bufs=1, space="PSUM") as psO,
            tc.tile_pool(name="psP", bufs=2, space="PSUM") as psP,
            tc.tile_pool(name="dramp", bufs=1, space="DRAM") as dramp,
        ):
            # ---------------- persistent tiles ----------------
            xT = const.tile([128, NKT, N], BF16)          # x^T (dim on partitions)
            wqsb = const.tile([128, NKT, GCOLS], BF16)
            wksb = const.tile([128, NKT, GCOLS], BF16)
            wvsb = const.tile([128, NKT, GCOLS], BF16)
            wosb = const.tile([128, 2, DIM], BF16)        # Wo rows, head-pair layout
            bosb = const.tile([128, DIM], F32)            # bo broadcast to 128 parts
            tmask = const.tile([128, 2, 128], BF16)       # triangular binary mask
            v1 = const.tile([128, NMT, HG, DH + 1], BF16)  # [V | ones]
            qth = const.tile([128, 2, N], BF16)           # Q^T head pairs
            kth = const.tile([128, 2, N], BF16)           # K^T head pairs
            ost = const.tile([65, HG, N], F32)            # unnormalized O^T + l row
            sel = const.tile([16, 16, 64], BF16)          # PE-broadcast selectors
            osb = const.tile([128, 2, N], BF16)           # normalized O^T, pair layout

            # ---------------- phase 1: loads (HWDGE, already bf16) ----------
            # wk first (the first projection emitted is K^T), then x per
            # k-tile so the K^T matmuls can start as soon as tiles land.
            nc.sync.dma_start(out=wksb[:, :, :],
                              in_=wk[:, :].rearrange("(t p) n -> p t n", p=128))
            for k in range(NKT):
                nc.sync.dma_start(out=xT[:, k, :], in_=xt[k * 128:(k + 1) * 128, :])
            for wsb, wdr in ((wqsb, wq), (wvsb, wv)):
                nc.sync.dma_start(
                    out=wsb[:, :, :],
                    in_=wdr[:, :].rearrange("(t p) n -> p t n", p=128),
                )
            for h in range(HG):
                p, e = divmod(h, 2)
                nc.sync.dma_start(out=wosb[e * 64:(e + 1) * 64, p, :],
                                  in_=wo[h * DH:(h + 1) * DH, :])
            bo_ap = bo[:]
            bo_bcast = bass.AP(tensor=bo_ap.tensor, offset=bo_ap.offset,
                               ap=[[0, 128]] + list(bo_ap.ap))
            nc.sync.dma_start(out=bosb[:, :], in_=bo_bcast)

            # ones column of [V | 1]
            nc.vector.memset(v1[:, :, :, DH:DH + 1], 1.0)
            # triangular binary mask for the 128-wide diagonal boundary
            # sub-block (identical for every diagonal block): keep q >= k,
            # i.e. f - p >= 0
            nc.gpsimd.memset(tmask[:, :, :], 1.0)
            nc.gpsimd.affine_select(
                out=tmask[:, :, :], in_=tmask[:, :, :],
                compare_op=mybir.AluOpType.is_ge, fill=0.0,
                base=0, pattern=[[0, 2], [1, 128]],
                channel_multiplier=-1,
            )
            # selector matrices for broadcasting softmax denominators across
            # partitions on the PE: sel[:, row, :].T @ r replicates r's row
            # `row` onto 64 output partitions
            nc.gpsimd.memset(sel[:, :, :], 1.0)
            nc.gpsimd.affine_select(
                out=sel[:, :, :], in_=sel[:, :, :],
                compare_op=mybir.AluOpType.is_equal, fill=0.0,
                base=0, pattern=[[-1, 16], [0, 64]],
                channel_multiplier=1,
            )

            # ---------- phases 2-4 fused: proj + attention per q-chunk ------
            def proj_chunk(c):
                # Q^T / K^T / V projections for chunk c's q range and k tiles
                cs = slice(c * 512, (c + 1) * 512)
                for dst, wsb in ((kth, wksb), (qth, wqsb)):
                    for pair in range(2):
                        pcols = slice(pair * 128, (pair + 1) * 128)
                        ps = psP.tile([128, 512], F32, tag="proj", name="ps_proj")
                        for k in range(NKT):
                            nc.tensor.matmul(ps[:, :], wsb[:, k, pcols], xT[:, k, cs],
                                             start=(k == 0), stop=(k == NKT - 1))
                        nc.vector.tensor_copy(out=dst[:, pair, cs], in_=ps[:, :])
                for mt in range(4 * c, 4 * c + 4):
                    ps = psP.tile([128, GCOLS], F32, tag="proj", name="ps_v")
                    for k in range(NKT):
                        nc.tensor.matmul(ps[:, :], xT[:, k, mt * 128:(mt + 1) * 128],
                                         wvsb[:, k, :],
                                         start=(k == 0), stop=(k == NKT - 1))
                    nc.vector.tensor_copy(
                        out=v1[:, mt, :, 0:DH],
                        in_=ps[:, :].rearrange("p (h d) -> p h d", h=HG),
                    )

            rdram = dramp.tile([64, 128], F32)

            def out_proj(c):
                # output projection (partial) for chunk c's sequence tiles
                for mt in range(4 * c, 4 * c + 4):
                    ms = slice(mt * 128, (mt + 1) * 128)
                    for nh in range(2):
                        ns = slice(nh * 512, (nh + 1) * 512)
                        ps = psP.tile([128, 512], F32, tag="proj", name="ps_out")
                        for p in range(2):
                            nc.tensor.matmul(ps[:, :], osb[:, p, ms], wosb[:, p, ns],
                                             start=(p == 0), stop=(p == 1))
                        ot = outs.tile([128, 512], BF16, tag="ot", name="ot")
                        nc.vector.tensor_add(ot[:, :], ps[:, :], bosb[:, ns])
                        nc.sync.dma_start(out=out[ms, ns], in_=ot[:, :])

            proj_chunk(0)
            attn_chunks = []

            def emit_rest_proj():
                for cc in range(1, NQC):
                    proj_chunk(cc)

            for c in range(NQC):
                if c == 1:
                    emit_rest_proj()
                qs = slice(c * 512, (c + 1) * 512)
                nkt = 4 * (c + 1)
                lc = lp.tile([16, 128], F32, tag="lc", name="lc")
                for pair in range(2):
                    hA, hB = 2 * pair, 2 * pair + 1
                    poA = psO.tile([65, 512], F32, tag="oA", name="poA")
                    poB = psO.tile([65, 512], F32, tag="oB", name="poB")
                    for k in range(nkt):
                        ks = slice(k * 128, (k + 1) * 128)
                        j = k - 4 * c
                        # on diagonal blocks, q columns f < 128*j are fully
                        # masked: skip their S^T stream, exp, and P@V
                        # accumulation entirely
                        fs = 128 * max(j, 0)
                        qsj = slice(c * 512 + fs, (c + 1) * 512)
                        ss = psS.tile([128, 1024], F32, tag="s", name="ss")
                        # S^T = K^T.T @ Q^T for both heads of the pair
                        # (row-tiled: head A rows 0:64, head B rows 64:128)
                        nc.tensor.matmul(ss[:, fs:512], kth[0:64, pair, ks],
                                         qth[0:64, pair, qsj],
                                         start=True, stop=True)
                        nc.tensor.matmul(ss[:, 512 + fs:1024], kth[64:128, pair, ks],
                                         qth[64:128, pair, qsj],
                                         start=True, stop=True)
                        pt = ptp.tile([128, 2, 512], BF16, tag="pt", name="pt")
                        if j < 0:
                            # below the diagonal: everything unmasked
                            nc.scalar.activation(out=pt[:, :, :], in_=ss[:, :],
                                                 func=mybir.ActivationFunctionType.Exp,
                                                 scale=SCALE)
                        else:
                            # diagonal block: exp the live columns, then zero
                            # the triangular boundary sub-block's upper part
                            nc.scalar.activation(
                                out=pt[:, :, fs:], in_=ss[:, :].rearrange(
                                    "p (e f) -> p e f", e=2)[:, :, fs:],
                                func=mybir.ActivationFunctionType.Exp,
                                scale=SCALE)
                            nc.vector.tensor_mul(pt[:, :, fs:fs + 128],
                                                 pt[:, :, fs:fs + 128],
                                                 tmask[:, :, :])
                        nc.tensor.matmul(poA[:, fs:], v1[:, k, hA, :],
                                         pt[:, 0, fs:],
                                         start=(k == 0), stop=(k == nkt - 1))
                        nc.tensor.matmul(poB[:, fs:], v1[:, k, hB, :],
                                         pt[:, 1, fs:],
                                         start=(k == 0), stop=(k == nkt - 1))
                    nc.vector.tensor_copy(out=ost[:, hA, qs], in_=poA[:, :])
                    nc.vector.tensor_copy(out=ost[:, hB, qs], in_=poB[:, :])
                    # stash both heads' l rows (partition 64) into the packed
                    # per-chunk l tile, split into 4 q-subblocks of 128 per
                    # head so the reciprocal spreads over 16 DVE lanes
                    for h in (hA, hB):
                        nc.sync.dma_start(
                            out=lc[4 * h: 4 * h + 4, :],
                            in_=ost[64:65, h, qs],
                        )

                # softmax denominators for this chunk, then normalize O^T.
                # For chunks that still have attention work behind them the
                # reciprocal rows bounce through DRAM (partition-broadcast
                # DMA, zero PE cost, latency hidden by the next chunk); the
                # final chunk's chain is exposed, so it broadcasts via tiny
                # selector matmuls on the PE instead (lower latency, and the
                # PE work fills the tail bubble).
                last = c == NQC - 1
                rc32 = lp.tile([16, 128], F32, tag="rc32", name="rc32")
                nc.vector.reciprocal(out=rc32[:, :], in_=lc[:, :])
                if last:
                    rc = lp.tile([16, 128], BF16, tag="rc", name="rc")
                    nc.vector.tensor_copy(out=rc[:, :], in_=rc32[:, :])
                else:
                    nc.sync.dma_start(out=rdram[c * 16:(c + 1) * 16, :],
                                      in_=rc32[:, :])
                    rb = lp.tile([64, 16, 128], F32, tag="rb", name="rb")
                    src = rdram[c * 16:(c + 1) * 16, :]
                    bcast = bass.AP(tensor=src.tensor, offset=src.offset,
                                    ap=[[0, 64]] + list(src.ap))
                    nc.sync.dma_start(out=rb[:, :, :], in_=bcast)
                for h in range(HG):
                    p, e = divmod(h, 2)
                    if last:
                        rbp = psP.tile([64, 512], F32, tag="proj", name="rb_ps")
                        for s in range(4):
                            nc.tensor.matmul(rbp[:, s * 128:(s + 1) * 128],
                                             sel[:, 4 * h + s, :], rc[:, :],
                                             start=True, stop=True)
                        r_src = rbp[:, :].rearrange("p (s f) -> p s f", f=128)
                    else:
                        r_src = rb[:, 4 * h: 4 * h + 4, :]
                    o_src = ost[0:64, h, qs].rearrange("p (s f) -> p s f", f=128)
                    if e == 0:
                        nc.vector.tensor_mul(
                            osb[0:64, p, qs].rearrange("p (s f) -> p s f", f=128),
                            o_src, r_src)
                    else:
                        # odd head lives on partitions 64:128 of the pair
                        # tensor; DVE can't cross partitions, so stage + DMA
                        onst = outs.tile([64, 512], BF16, tag="onst", name="onst")
                        nc.vector.tensor_mul(
                            onst[:, :].rearrange("p (s f) -> p s f", f=128),
                            o_src, r_src)
                        nc.sync.dma_start(out=osb[64:128, p, qs], in_=onst[:, :])

                # out-proj lags one chunk behind so its PE work can fill the
                # normalize-chain bubble of the final chunk
                if c >= 1:
                    out_proj(c - 1)
            out_proj(NQC - 1)

    nc.compile()
    return nc


def _in_maps(inputs):
    bf = ml_dtypes.bfloat16
    x = np.asarray(inputs["x"], np.float32)
    Wq = np.asarray(inputs["Wq"], np.float32).astype(bf)
    Wkv = np.asarray(inputs["Wkv"], np.float32).astype(bf)
    Wo = np.asarray(inputs["Wo"], np.float32).astype(bf)
    bo = np.asarray(inputs["bo"], np.float32)
    maps = []
    for i in range(8):
        b, g = divmod(i, GROUPS)
        cs = slice(g * GCOLS, (g + 1) * GCOLS)
        maps.append(dict(
            xt=np.ascontiguousarray(x[b].T.astype(bf)),
            wq=np.ascontiguousarray(Wq[:, cs]),
            wk=np.ascontiguousarray(Wkv[:, cs]),
            wv=np.ascontiguousarray(Wkv[:, DIM + g * GCOLS: DIM + (g + 1) * GCOLS]),
            wo=np.ascontiguousarray(Wo[cs, :]),
            bo=np.ascontiguousarray(bo / GROUPS),
        ))
    return maps


_NC = None


def _get_nc():
    global _NC
    if _NC is None:
        nc = build()
        nc.finalize()
        _NC = nc
    return _NC


def run(inputs, trace=False, **kwargs):
    maps = _in_maps(inputs)
    res = run_bass_kernel_spmd(_get_nc(), maps, core_ids=list(range(8)),
                               trace=trace, **kwargs)
    out = np.empty((B, N, DIM), np.float32)
    for b in range(B):
        acc = res.results[4 * b]["out"].astype(np.float32)
        for g in range(1, GROUPS):
            acc = acc + res.results[4 * b + g]["out"].astype(np.float32)
        out[b] = acc
    return out, res


def kernel(**inputs):
    out, _ = run(inputs, trace=False)
    return out
